# revision 38
# baseline (speedup 1.0000x reference)
"""Trainium2 Bass kernel for a 16-head causal self-attention block.

Reference computation (B=1, S=4096, H=2048, 16 heads x 128 dim, fp32):
    qkv = x @ w_qkv.T            # [S, 6144]
    q, k = rope(q), rope(k)      # half-split rope
    attn = causal_softmax(q k^T / sqrt(128)) @ v
    out  = attn @ w_o.T          # [S, 2048]

Sharding: tensor-parallel over heads.  Each of the 8 cores owns 2 heads:
it computes its slice of the QKV projection (768 rows), attention for its
2 heads, and a partial o_proj ([S, 2048] bf16); the host sums the 8
partials in fp32.

Single fused loop, iteration t = s-tile t (512 rows) + attention q-tile t:
the QKV sweeps (PE-heavy, ACT/DVE-light) interleave with the attention
chunk loop (ACT exp paced at ~1.1us/chunk vs PE 0.85us/chunk), so no
engine starves.  o_proj matmuls of tile t-1 are woven into tile t's chunk
loop to fill the PE while ACT paces.  Softmax denominator accumulation
alternates DVE (even chunks) and GPSIMD (odd chunks).  Eight PSUM banks
are shared by role:
  S0 [128,1024]: V-sweep psum   -> score chunks (even)
  S1 [128,1024]: Q-sweep psum   -> score chunks (odd)
  Pa/Pb [128,512]: attention PV accumulators (head 0 / head 1)
  Oa/Ob [128,512]: rope-q rotate -> o_proj ring -> K-sweep psum
                   -> rope-k rotate -> softmax-denominator fold
"""

import numpy as np

import concourse.bass as bass
import concourse.mybir as mybir
import concourse.tile as tile
from concourse import bacc
from concourse.bass_utils import run_bass_kernel_spmd

F32 = mybir.dt.float32
F32R = mybir.dt.float32r
BF16 = mybir.dt.bfloat16

S = 4096
H = 2048
DH = 128
NH = 16
NCORES = 8
HPC = NH // NCORES          # 2 heads per core
OLOC = HPC * DH             # 256 local o-channels per q/k/v group
P = 128
ST1 = 512                   # s-tile / q-tile width
NT = S // ST1               # 8 fused iterations
NHT = H // P                # 16 h-chunks of the hidden dim
SCALE = 1.0 / float(np.sqrt(np.float32(DH)))

_PROGRAM = None


def _build_body(tc):
    nc = tc.nc

    xT = nc.dram_tensor("xT", [H, S], BF16, kind="ExternalInput").ap()
    wqkvT = nc.dram_tensor("wqkvT", [H, 3 * OLOC], BF16, kind="ExternalInput").ap()
    woT = nc.dram_tensor("woT", [OLOC, H], BF16, kind="ExternalInput").ap()
    rope = nc.dram_tensor("rope", [P, 2, S], F32, kind="ExternalInput").ap()
    swapj = nc.dram_tensor("swapj", [P, P], BF16, kind="ExternalInput").ap()
    onesin = nc.dram_tensor("onesin", [P, P], F32R, kind="ExternalInput").ap()
    masks = nc.dram_tensor("masks", [4, P, 2 * ST1], BF16, kind="ExternalInput").ap()
    out = nc.dram_tensor("out", [S, H], BF16, kind="ExternalOutput").ap()

    xT_v = xT.rearrange("(t p) s -> p t s", p=P)        # [128, 16, 4096]
    wq_v = wqkvT.rearrange("(t p) o -> p t o", p=P)     # [128, 16, 768]
    woT_v = woT.rearrange("(t p) h -> p t h", p=P)      # [128, 2, 2048]

    with (
        tc.tile_pool(name="res", bufs=1) as res,
        tc.tile_pool(name="xp", bufs=1) as xp,
        tc.tile_pool(name="tabp", bufs=2) as tabp,
        tc.tile_pool(name="tmpp", bufs=4) as tmpp,
        tc.tile_pool(name="ep", bufs=6) as ep,
        tc.tile_pool(name="accp", bufs=2) as accp,
        tc.tile_pool(name="recp", bufs=2) as recp,
        tc.tile_pool(name="stgp", bufs=5) as stgp,
        tc.tile_pool(name="ps", bufs=1, space="PSUM") as ps,
    ):
        # ---- resident SBUF tensors ----
        QT_sb = res.tile([P, HPC, S], BF16)     # d-major roped Q^T
        KT_sb = res.tile([P, HPC, S], BF16)     # d-major roped K^T
        V_sb = res.tile([P, S // P, OLOC], BF16)  # s-major V
        A_sb = res.tile([P, HPC, S], BF16)      # normalized attn^T
        wT_sb = res.tile([P, NHT, 3 * OLOC], BF16)
        woT_sb = res.tile([P, HPC, H], BF16)
        J_sb = res.tile([P, P], BF16)
        ones_sb = res.tile([P, P], F32R)
        ones_bf = res.tile([P, P], BF16)
        masks_sb = res.tile([P, 4, 2 * ST1], BF16)

        # ---- PSUM banks (8 x 2KiB), manually role-shared ----
        # Two-deep f32 score ring; Pa/Pb double as the V-sweep psum;
        # Wa/Wb cycle Q -> rope-q -> o_proj -> K -> rope-k -> fold.
        SC4 = [ps.tile([P, 2 * ST1], F32, tag=f"sc{i}", name=f"SC{i}")
               for i in range(2)]
        Pa = ps.tile([P, ST1], F32, tag="pa", name="Pa")
        Pb = ps.tile([P, ST1], F32, tag="pb", name="Pb")
        Wa = ps.tile([P, ST1], F32, tag="wa", name="Wa")
        Wb = ps.tile([P, ST1], F32, tag="wb", name="Wb")
        W2 = [Wa, Wb]
        PV2 = [Pa, Pb]

        # ---- startup DMAs. Per-DMA ring overhead (~0.6us) dominates small
        # transfers, so: x-tiles of s-tile 0 stream individually on the
        # scalar ring (pipelined arrival for the first V matmuls) while the
        # sync ring carries the V-column weight slice first, then the rest;
        # later s-tiles arrive as ONE bulk DMA each.
        xts = {}
        for ht in range(NHT):
            xt0 = xp.tile([P, ST1], BF16, tag="xt0", name=f"xt0_{ht}",
                          bufs=NHT)
            nc.scalar.dma_start(xt0, xT_v[:, ht, 0:ST1])
            xts[(0, ht)] = xt0
        nc.sync.dma_start(wT_sb[:, :, 2 * OLOC:3 * OLOC],
                          wq_v[:, :, 2 * OLOC:3 * OLOC])
        nc.sync.dma_start(wT_sb[:, :, 0:2 * OLOC], wq_v[:, :, 0:2 * OLOC])
        nc.sync.dma_start(ones_sb, onesin)
        tab0 = tabp.tile([P, 2, ST1], F32, tag="tab", name="tab0")
        nc.scalar.dma_start(tab0, rope[:, :, 0:ST1])
        tabs = {0: tab0}
        nc.scalar.dma_start(J_sb, swapj)
        nc.gpsimd.memset(ones_bf, 1.0)
        nc.scalar.dma_start(masks_sb, masks.rearrange("j p q -> p j q"))
        nc.scalar.dma_start(woT_sb, woT_v)
        xbulk = {}

        def xfetch(t, ht):
            if t == 0:
                return xts[(0, ht)]
            return xbulk[t][:, ht, :]

        def prefetch(t):
            if t >= NT or t in xbulk:
                return
            xb = xp.tile([P, NHT, ST1], BF16, tag="xb", name=f"xb{t}",
                         bufs=2)
            nc.sync.dma_start(xb, xT_v[:, :, t * ST1:(t + 1) * ST1])
            xbulk[t] = xb
            tab = tabp.tile([P, 2, ST1], F32, tag="tab", name=f"tab{t}")
            nc.sync.dma_start(tab, rope[:, :, t * ST1:(t + 1) * ST1])
            tabs[t] = tab

        def rope_muls(t, which, h, blk):
            # t1 = blk*cos, t2 = blk*sin (rotate via J matmul happens later;
            # the sin table rows are doubled so sin can pre-multiply)
            cos = tabs[t][:, 0, :]
            sin = tabs[t][:, 1, :]
            t1 = tmpp.tile([P, ST1], F32, tag="t1", name=f"t1{which}{t}_{h}")
            t2 = tmpp.tile([P, ST1], BF16, tag="t2", name=f"t2{which}{t}_{h}")
            nc.vector.tensor_mul(t1, blk, cos)
            nc.vector.tensor_mul(t2, blk, sin)
            return t1, t2

        def rope_finish(t, h, t1, t2, dst_sb):
            rot = W2[h]
            nc.tensor.matmul(rot, lhsT=J_sb, rhs=t2, start=True, stop=True)
            nc.vector.tensor_add(dst_sb[:, h, t * ST1:(t + 1) * ST1], t1, rot)

        class ChunkStream:
            """Attention chunk pipeline for q-tile tq: score matmuls one
            chunk ahead of PV, exp on ACT, deferrable denominator adds."""

            def __init__(self, tq):
                self.tq = tq
                self.nch = 4 * tq + 4
                self.c = 0
                self.pend = []          # (c, e) with pv not yet emitted
                self.acc_backlog = []   # (c, e) with acc-add not yet emitted
                self.acc2 = [
                    accp.tile([P, 2 * ST1], F32R if a == 0 else BF16,
                              tag=f"acc{a}", name=f"acc{a}_{tq}")
                    for a in range(2)
                ]

            @property
            def more(self):
                return self.c < self.nch

            def _acc(self, c, e):
                acc = self.acc2[c % 2]
                if c < 2:
                    nc.vector.tensor_copy(acc, e)
                else:
                    nc.vector.tensor_add(acc, acc, e)

            def drain_acc(self):
                for c, e in self.acc_backlog:
                    self._acc(c, e)
                self.acc_backlog = []

            def step(self, defer_acc=False):
                if not self.more:
                    return
                tq, c = self.tq, self.c
                sc = SC4[c % 2]
                for h in range(HPC):
                    nc.tensor.matmul(
                        sc[:, h * ST1:(h + 1) * ST1],
                        lhsT=KT_sb[:, h, c * P:(c + 1) * P],
                        rhs=QT_sb[:, h, tq * ST1:(tq + 1) * ST1],
                        start=True, stop=True,
                    )
                e = ep.tile([P, 2 * ST1], BF16, tag="e", name=f"e{tq}_{c}")
                nc.scalar.activation(
                    e, sc, mybir.ActivationFunctionType.Exp, scale=SCALE
                )
                if c >= 4 * tq:
                    nc.vector.tensor_mul(e, e, masks_sb[:, c - 4 * tq, :])
                if defer_acc:
                    self.acc_backlog.append((c, e))
                else:
                    self.drain_acc()
                    self._acc(c, e)
                self.pend.append((c, e))
                if len(self.pend) > 1:
                    self._pv(*self.pend.pop(0))
                self.c += 1

            def _pv(self, c, e):
                for h in range(HPC):
                    nc.tensor.matmul(
                        PV2[h],
                        lhsT=V_sb[:, c, h * P:(h + 1) * P],
                        rhs=e[:, h * ST1:(h + 1) * ST1],
                        start=(c == 0), stop=(c == self.nch - 1),
                    )

            def flush_pv(self):
                if self.pend:
                    self._pv(*self.pend.pop(0))

            def finish(self):
                while self.more:
                    self.step()
                for c, e in self.pend:
                    self._pv(c, e)
                self.pend = []
                self.drain_acc()

            def fold_normalize(self):
                self.finish()
                for h in range(HPC):
                    for a in range(2):
                        nc.tensor.matmul(
                            W2[h],
                            lhsT=ones_sb if a == 0 else ones_bf,
                            rhs=self.acc2[a][:, h * ST1:(h + 1) * ST1],
                            start=(a == 0), stop=(a == 1),
                        )
                for h in range(HPC):
                    rec = recp.tile([P, ST1], F32, tag="rec",
                                    name=f"rec{self.tq}_{h}")
                    nc.vector.reciprocal_approx_fast(rec, W2[h])
                    nc.vector.tensor_mul(
                        A_sb[:, h, self.tq * ST1:(self.tq + 1) * ST1],
                        PV2[h], rec,
                    )

        def po_emitter(t):
            """Yield o_proj work for q-tile t: 16 matmul-pairs + stg + dma."""
            if t < 0:
                return
            for sub in range(ST1 // P):
                i = t * (ST1 // P) + sub
                for htile in range(H // ST1):
                    g = sub * (H // ST1) + htile
                    po = W2[g % 2]
                    for oc in range(HPC):
                        nc.tensor.matmul(
                            po,
                            lhsT=A_sb[:, oc, i * P:(i + 1) * P],
                            rhs=woT_sb[:, oc, htile * ST1:(htile + 1) * ST1],
                            start=(oc == 0), stop=(oc == HPC - 1),
                        )
                    stg = stgp.tile([P, ST1], BF16, tag="stg",
                                    name=f"stg{t}_{g}")
                    # keep stg off the ACT queue: exps pace the chunk
                    # pipeline and must not sit behind staging copies
                    if t == NT - 1 and g % 2 == 1:
                        nc.scalar.activation(
                            stg, po, mybir.ActivationFunctionType.Copy
                        )
                    else:
                        nc.vector.tensor_copy(stg, po)
                    nc.sync.dma_start(
                        out[i * P:(i + 1) * P, htile * ST1:(htile + 1) * ST1],
                        stg,
                    )
                    yield

        # ---------------- fused iteration loop ----------------
        # Iteration t: V sweep (P banks) -> Q sweep + rope (W banks) ->
        # attention part A for q-tile t (old K/V chunks) with o_proj of
        # q-tile t-1 woven in -> K sweep + rope (W banks) -> diagonal
        # chunks -> fold/normalize.
        cs_prev = None
        for t in range(NT):
            s0c = t * (ST1 // P)        # first V/K chunk index of this tile
            prefetch(t + 1)             # 36-deep x ring: safe this early
            po_it = po_emitter(t - 1)
            po_left = 16 if t >= 1 else 0

            def take_po(n):
                nonlocal po_left
                for _ in range(min(n, po_left)):
                    next(po_it)
                    po_left -= 1

            # --- V sweep into the even score tile (its last reader, an exp
            #     two chunks back, is long done); epilogue copies into V_sb
            for ht in range(NHT):
                for sub in range(ST1 // P):
                    nc.tensor.matmul(
                        SC4[0][:, sub * OLOC:(sub + 1) * OLOC],
                        lhsT=xfetch(t, ht)[:, sub * P:(sub + 1) * P],
                        rhs=wT_sb[:, ht, 2 * OLOC:3 * OLOC],
                        start=(ht == 0) and sub % 2 == 0,
                        stop=(ht == NHT - 1) and sub % 2 == 1,
                    )
            for sub in range(ST1 // P):
                nc.scalar.activation(
                    V_sb[:, s0c + sub, :],
                    SC4[0][:, sub * OLOC:(sub + 1) * OLOC],
                    mybir.ActivationFunctionType.Copy,
                )

            # fold/normalize of the previous q-tile lands here: the V sweep
            # above hid the DVE denominator-chain drain, and the reciprocal
            # finishes under the Q sweep below
            if cs_prev is not None:
                cs_prev.fold_normalize()
                cs_prev = None

            # --- Q sweep into Wa/Wb (head-major, rope muls under the other
            #     head's sweep); rope into QT_sb ---
            q12 = [None, None]
            for h in range(HPC):
                for ht in range(NHT):
                    nc.tensor.matmul(
                        W2[h],
                        lhsT=wT_sb[:, ht, h * P:(h + 1) * P],
                        rhs=xfetch(t, ht),
                        start=ht == 0, stop=ht == NHT - 1,
                    )
                q12[h] = rope_muls(t, "q", h, W2[h])
            rope_finish(t, 0, q12[0][0], q12[0][1], QT_sb)
            take_po(2)
            rope_finish(t, 1, q12[1][0], q12[1][1], QT_sb)
            take_po(2)

            # --- part A: chunks over already-built K/V, o_proj woven in ---
            cs = ChunkStream(t)
            for c in range(4 * t):
                cs.step()
                if po_left > 4:
                    take_po(2 if c % 2 == 0 else 1)

            # --- K sweep into Wa/Wb (head-major); rope into KT_sb ---
            take_po(po_left - 4)        # drain surplus o_proj, keep 4
            k12 = [None, None]
            for h in range(HPC):
                for ht in range(NHT):
                    nc.tensor.matmul(
                        W2[h],
                        lhsT=wT_sb[:, ht, OLOC + h * P:OLOC + (h + 1) * P],
                        rhs=xfetch(t, ht),
                        start=ht == 0, stop=ht == NHT - 1,
                    )
                k12[h] = rope_muls(t, "k", h, W2[h])
            cs.flush_pv()
            rope_finish(t, 0, k12[0][0], k12[0][1], KT_sb)
            cs.flush_pv()
            rope_finish(t, 1, k12[1][0], k12[1][1], KT_sb)
            cs.flush_pv()

            # --- part B: diagonal chunks, reserved o_proj as fill ---
            while cs.more:
                cs.step()
                take_po(1)
            cs.finish()
            take_po(po_left)
            cs_prev = cs

        cs_prev.fold_normalize()
        # final o_proj for tile NT-1
        for _ in po_emitter(NT - 1):
            pass


def build_program():
    """Build + compile the Bass program (same program for all 8 cores)."""
    global _PROGRAM
    if _PROGRAM is not None:
        return _PROGRAM
    nc = bacc.Bacc(
        "TRN2", target_bir_lowering=False, debug=False, enable_asserts=False
    )
    with tile.TileContext(nc) as tc:
        _build_body(tc)
    nc.compile()
    _PROGRAM = nc
    return nc


def make_in_maps(hidden_states, w_qkv, w_o):
    import ml_dtypes

    x = np.asarray(hidden_states, dtype=np.float32).reshape(S, H)
    w = np.asarray(w_qkv, dtype=np.float32)
    wo = np.asarray(w_o, dtype=np.float32)

    xT = np.ascontiguousarray(x.T).astype(ml_dtypes.bfloat16)    # [2048, 4096]

    # rope tables, [128, 2, 4096]: rows 0:64 and 64:128 both hold the
    # [64, S] table so the doubled layout lines up with [real; imag] dims.
    e = np.arange(0, DH, 2, dtype=np.float32) / np.float32(DH)
    inv_freq = (1.0 / np.power(np.float32(10000.0), e)).astype(np.float32)
    t = np.arange(S, dtype=np.float32)
    freqs = np.outer(t, inv_freq).astype(np.float32)     # [S, 64]
    cosT = np.cos(freqs).T                               # [64, S]
    sinT = np.sin(freqs).T
    rope = np.empty((P, 2, S), dtype=np.float32)
    rope[0:64, 0] = cosT
    rope[64:128, 0] = cosT
    rope[0:64, 1] = sinT
    rope[64:128, 1] = sinT

    # signed half-swap permutation: (J.T @ z)[d] = -z[64+d], [64+d] = +z[d]
    swapj = np.zeros((P, P), dtype=ml_dtypes.bfloat16)
    for d in range(64):
        swapj[64 + d, d] = -1.0
        swapj[d, 64 + d] = 1.0

    # diagonal-block masks [4, 128, 1024]: chunk at k0 = q0 + 128j keeps
    # (ki, qi) iff qi >= ki + 128j; tiled twice along q for the 2-head tile.
    ki = np.arange(P)[:, None]
    qi = np.arange(ST1)[None, :]
    masks = np.empty((4, P, 2 * ST1), dtype=ml_dtypes.bfloat16)
    for j in range(4):
        m = (qi >= ki + 128 * j).astype(ml_dtypes.bfloat16)
        masks[j] = np.concatenate([m, m], axis=1)

    in_maps = []
    for c in range(NCORES):
        r0 = c * OLOC
        w_loc = np.concatenate(
            [
                w[r0:r0 + OLOC],
                w[NH * DH + r0:NH * DH + r0 + OLOC],
                w[2 * NH * DH + r0:2 * NH * DH + r0 + OLOC],
            ],
            axis=0,
        )                                                # [768, 2048]
        wqkvT_c = np.ascontiguousarray(w_loc.T).astype(ml_dtypes.bfloat16)
        woT_c = np.ascontiguousarray(
            wo[:, r0:r0 + OLOC].T
        ).astype(ml_dtypes.bfloat16)                     # [256, 2048]
        in_maps.append(
            {
                "xT": xT,
                "wqkvT": wqkvT_c,
                "woT": woT_c,
                "rope": rope,
                "swapj": swapj,
                "onesin": np.ones((P, P), dtype=np.float32),
                "masks": masks,
            }
        )
    return in_maps


def run_cores(in_maps, trace=False, **kwargs):
    nc = build_program()
    return run_bass_kernel_spmd(
        nc, in_maps, list(range(NCORES)), trace=trace, **kwargs
    )


def kernel(hidden_states, w_qkv, w_o):
    in_maps = make_in_maps(hidden_states, w_qkv, w_o)
    res = run_cores(in_maps)
    acc = res.results[0]["out"].astype(np.float32)
    for c in range(1, NCORES):
        acc = acc + res.results[c]["out"].astype(np.float32)
    return acc.reshape(1, S, H)


# revision 39
# speedup vs baseline: 1.2032x; 1.2032x over previous
"""Trainium2 Bass kernel for a 16-head causal self-attention block.

Reference computation (B=1, S=4096, H=2048, 16 heads x 128 dim, fp32):
    qkv = x @ w_qkv.T            # [S, 6144]
    q, k = rope(q), rope(k)      # half-split rope
    attn = causal_softmax(q k^T / sqrt(128)) @ v
    out  = attn @ w_o.T          # [S, 2048]

Sharding: tensor-parallel over heads.  Each of the 8 cores owns 2 heads:
it computes its slice of the QKV projection (768 rows), attention for its
2 heads, and a partial o_proj ([S, 2048] bf16); the host sums the 8
partials in fp32.

Single fused loop, iteration t = s-tile t (512 rows) + attention q-tile t:
the QKV sweeps (PE-heavy, ACT/DVE-light) interleave with the attention
chunk loop (ACT exp paced at ~1.1us/chunk vs PE 0.85us/chunk), so no
engine starves.  o_proj matmuls of tile t-1 are woven into tile t's chunk
loop to fill the PE while ACT paces.  Softmax denominator accumulation
alternates DVE (even chunks) and GPSIMD (odd chunks).  Eight PSUM banks
are shared by role:
  S0 [128,1024]: V-sweep psum   -> score chunks (even)
  S1 [128,1024]: Q-sweep psum   -> score chunks (odd)
  Pa/Pb [128,512]: attention PV accumulators (head 0 / head 1)
  Oa/Ob [128,512]: rope-q rotate -> o_proj ring -> K-sweep psum
                   -> rope-k rotate -> softmax-denominator fold
"""

import numpy as np

import concourse.bass as bass
import concourse.mybir as mybir
import concourse.tile as tile
from concourse import bacc
from concourse.bass_utils import run_bass_kernel_spmd

F32 = mybir.dt.float32
F32R = mybir.dt.float32r
BF16 = mybir.dt.bfloat16

S = 4096
H = 2048
DH = 128
NH = 16
NCORES = 8
HPC = NH // NCORES          # 2 heads per core
OLOC = HPC * DH             # 256 local o-channels per q/k/v group
P = 128
ST1 = 512                   # s-tile / q-tile width
NT = S // ST1               # 8 fused iterations
NHT = H // P                # 16 h-chunks of the hidden dim
SCALE = 1.0 / float(np.sqrt(np.float32(DH)))

_PROGRAM = None


def _build_body(tc):
    nc = tc.nc

    xT = nc.dram_tensor("xT", [H, S], BF16, kind="ExternalInput").ap()
    wqkvT = nc.dram_tensor("wqkvT", [H, 3 * OLOC], BF16, kind="ExternalInput").ap()
    woT = nc.dram_tensor("woT", [OLOC, H], BF16, kind="ExternalInput").ap()
    rope = nc.dram_tensor("rope", [P, 2, S], F32, kind="ExternalInput").ap()
    swapj = nc.dram_tensor("swapj", [P, P], BF16, kind="ExternalInput").ap()
    onesin = nc.dram_tensor("onesin", [P, P], F32R, kind="ExternalInput").ap()
    masks = nc.dram_tensor("masks", [4, P, 2 * ST1], BF16, kind="ExternalInput").ap()
    out = nc.dram_tensor("out", [S, H], BF16, kind="ExternalOutput").ap()

    xT_v = xT.rearrange("(t p) s -> p t s", p=P)        # [128, 16, 4096]
    wq_v = wqkvT.rearrange("(t p) o -> p t o", p=P)     # [128, 16, 768]
    woT_v = woT.rearrange("(t p) h -> p t h", p=P)      # [128, 2, 2048]

    with (
        tc.tile_pool(name="res", bufs=1) as res,
        tc.tile_pool(name="xp", bufs=1) as xp,
        tc.tile_pool(name="tabp", bufs=2) as tabp,
        tc.tile_pool(name="tmpp", bufs=4) as tmpp,
        tc.tile_pool(name="ep", bufs=6) as ep,
        tc.tile_pool(name="accp", bufs=2) as accp,
        tc.tile_pool(name="recp", bufs=2) as recp,
        tc.tile_pool(name="stgp", bufs=5) as stgp,
        tc.tile_pool(name="ps", bufs=1, space="PSUM") as ps,
    ):
        # ---- resident SBUF tensors ----
        QT_sb = res.tile([P, HPC, S], BF16)     # d-major roped Q^T
        KT_sb = res.tile([P, HPC, S], BF16)     # d-major roped K^T
        V_sb = res.tile([P, S // P, OLOC], BF16)  # s-major V
        A_sb = res.tile([P, HPC, S], BF16)      # normalized attn^T
        wT_sb = res.tile([P, NHT, 3 * OLOC], BF16)
        woT_sb = res.tile([P, HPC, H], BF16)
        J_sb = res.tile([P, P], BF16)
        ones_sb = res.tile([P, P], F32R)
        ones_bf = res.tile([P, P], BF16)
        masks_sb = res.tile([P, 4, 2 * ST1], BF16)

        # ---- PSUM banks (8 x 2KiB), manually role-shared ----
        # Two-deep f32 score ring; Pa/Pb double as the V-sweep psum;
        # Wa/Wb cycle Q -> rope-q -> o_proj -> K -> rope-k -> fold.
        SC4 = [ps.tile([P, 2 * ST1], F32, tag=f"sc{i}", name=f"SC{i}")
               for i in range(2)]
        Pa = ps.tile([P, ST1], F32, tag="pa", name="Pa")
        Pb = ps.tile([P, ST1], F32, tag="pb", name="Pb")
        Wa = ps.tile([P, ST1], F32, tag="wa", name="Wa")
        Wb = ps.tile([P, ST1], F32, tag="wb", name="Wb")
        W2 = [Wa, Wb]
        PV2 = [Pa, Pb]

        # ---- startup DMAs. Per-DMA ring overhead (~0.6us) dominates small
        # transfers, so: x-tiles of s-tile 0 stream individually on the
        # scalar ring (pipelined arrival for the first V matmuls) while the
        # sync ring carries the V-column weight slice first, then the rest;
        # later s-tiles arrive as ONE bulk DMA each.
        xts = {}
        for ht in range(NHT):
            xt0 = xp.tile([P, ST1], BF16, tag="xt0", name=f"xt0_{ht}",
                          bufs=NHT)
            nc.scalar.dma_start(xt0, xT_v[:, ht, 0:ST1])
            xts[(0, ht)] = xt0
        nc.sync.dma_start(wT_sb[:, :, 2 * OLOC:3 * OLOC],
                          wq_v[:, :, 2 * OLOC:3 * OLOC])
        nc.sync.dma_start(wT_sb[:, :, 0:2 * OLOC], wq_v[:, :, 0:2 * OLOC])
        nc.sync.dma_start(ones_sb, onesin)
        tab0 = tabp.tile([P, 2, ST1], F32, tag="tab", name="tab0")
        nc.scalar.dma_start(tab0, rope[:, :, 0:ST1])
        tabs = {0: tab0}
        nc.scalar.dma_start(J_sb, swapj)
        nc.gpsimd.memset(ones_bf, 1.0)
        nc.scalar.dma_start(masks_sb, masks.rearrange("j p q -> p j q"))
        nc.scalar.dma_start(woT_sb, woT_v)
        xbulk = {}

        def xfetch(t, ht):
            if t == 0:
                return xts[(0, ht)]
            return xbulk[t][:, ht, :]

        def prefetch(t):
            if t >= NT or t in xbulk:
                return
            xb = xp.tile([P, NHT, ST1], BF16, tag="xb", name=f"xb{t}",
                         bufs=2)
            nc.sync.dma_start(xb, xT_v[:, :, t * ST1:(t + 1) * ST1])
            xbulk[t] = xb
            tab = tabp.tile([P, 2, ST1], F32, tag="tab", name=f"tab{t}")
            nc.sync.dma_start(tab, rope[:, :, t * ST1:(t + 1) * ST1])
            tabs[t] = tab

        def rope_muls(t, which, h, blk):
            # t1 = blk*cos, t2 = blk*sin (rotate via J matmul happens later;
            # the sin table rows are doubled so sin can pre-multiply)
            cos = tabs[t][:, 0, :]
            sin = tabs[t][:, 1, :]
            t1 = tmpp.tile([P, ST1], F32, tag="t1", name=f"t1{which}{t}_{h}")
            t2 = tmpp.tile([P, ST1], BF16, tag="t2", name=f"t2{which}{t}_{h}")
            nc.vector.tensor_mul(t1, blk, cos)
            nc.vector.tensor_mul(t2, blk, sin)
            return t1, t2

        def rope_finish(t, h, t1, t2, dst_sb):
            rot = W2[h]
            nc.tensor.matmul(rot, lhsT=J_sb, rhs=t2, start=True, stop=True)
            nc.vector.tensor_add(dst_sb[:, h, t * ST1:(t + 1) * ST1], t1, rot)

        class ChunkStream:
            """Attention chunk pipeline for q-tile tq: score matmuls one
            chunk ahead of PV, exp on ACT, deferrable denominator adds."""

            def __init__(self, tq):
                self.tq = tq
                self.nch = 4 * tq + 4
                self.c = 0
                self.pend = []          # (c, e) with pv not yet emitted
                self.acc_backlog = []   # (c, e) with acc-add not yet emitted
                self.acc2 = [
                    accp.tile([P, 2 * ST1], F32R if a == 0 else BF16,
                              tag=f"acc{a}", name=f"acc{a}_{tq}")
                    for a in range(2)
                ]

            @property
            def more(self):
                return self.c < self.nch

            def _acc(self, c, e):
                acc = self.acc2[c % 2]
                if c < 2:
                    nc.vector.tensor_copy(acc, e)
                else:
                    nc.vector.tensor_add(acc, acc, e)

            def drain_acc(self):
                for c, e in self.acc_backlog:
                    self._acc(c, e)
                self.acc_backlog = []

            def step(self, defer_acc=False):
                if not self.more:
                    return
                tq, c = self.tq, self.c
                sc = SC4[c % 2]
                for h in range(HPC):
                    nc.tensor.matmul(
                        sc[:, h * ST1:(h + 1) * ST1],
                        lhsT=KT_sb[:, h, c * P:(c + 1) * P],
                        rhs=QT_sb[:, h, tq * ST1:(tq + 1) * ST1],
                        start=True, stop=True,
                    )
                e = ep.tile([P, 2 * ST1], BF16, tag="e", name=f"e{tq}_{c}")
                nc.scalar.activation(
                    e, sc, mybir.ActivationFunctionType.Exp, scale=SCALE
                )
                if c >= 4 * tq:
                    nc.vector.tensor_mul(e, e, masks_sb[:, c - 4 * tq, :])
                if defer_acc:
                    self.acc_backlog.append((c, e))
                else:
                    self.drain_acc()
                    self._acc(c, e)
                self.pend.append((c, e))
                if len(self.pend) > 1:
                    self._pv(*self.pend.pop(0))
                self.c += 1

            def _pv(self, c, e):
                for h in range(HPC):
                    nc.tensor.matmul(
                        PV2[h],
                        lhsT=V_sb[:, c, h * P:(h + 1) * P],
                        rhs=e[:, h * ST1:(h + 1) * ST1],
                        start=(c == 0), stop=(c == self.nch - 1),
                    )

            def flush_pv(self):
                if self.pend:
                    self._pv(*self.pend.pop(0))

            def finish(self):
                while self.more:
                    self.step()
                for c, e in self.pend:
                    self._pv(c, e)
                self.pend = []
                self.drain_acc()

            def fold_normalize(self):
                self.finish()
                for h in range(HPC):
                    for a in range(2):
                        nc.tensor.matmul(
                            W2[h],
                            lhsT=ones_sb if a == 0 else ones_bf,
                            rhs=self.acc2[a][:, h * ST1:(h + 1) * ST1],
                            start=(a == 0), stop=(a == 1),
                        )
                for h in range(HPC):
                    rec = recp.tile([P, ST1], F32, tag="rec",
                                    name=f"rec{self.tq}_{h}")
                    nc.vector.reciprocal_approx_fast(rec, W2[h])
                    nc.vector.tensor_mul(
                        A_sb[:, h, self.tq * ST1:(self.tq + 1) * ST1],
                        PV2[h], rec,
                    )

        def po_emitter(t):
            """Yield o_proj work for q-tile t: 16 matmul-pairs + stg + dma."""
            if t < 0:
                return
            for sub in range(ST1 // P):
                i = t * (ST1 // P) + sub
                for htile in range(H // ST1):
                    g = sub * (H // ST1) + htile
                    po = W2[g % 2]
                    for oc in range(HPC):
                        nc.tensor.matmul(
                            po,
                            lhsT=A_sb[:, oc, i * P:(i + 1) * P],
                            rhs=woT_sb[:, oc, htile * ST1:(htile + 1) * ST1],
                            start=(oc == 0), stop=(oc == HPC - 1),
                        )
                    stg = stgp.tile([P, ST1], BF16, tag="stg",
                                    name=f"stg{t}_{g}")
                    # keep stg off the ACT queue: exps pace the chunk
                    # pipeline and must not sit behind staging copies
                    if t == NT - 1 and g % 2 == 1:
                        nc.scalar.activation(
                            stg, po, mybir.ActivationFunctionType.Copy
                        )
                    else:
                        nc.vector.tensor_copy(stg, po)
                    nc.sync.dma_start(
                        out[i * P:(i + 1) * P, htile * ST1:(htile + 1) * ST1],
                        stg,
                    )
                    yield

        # ---------------- fused iteration loop ----------------
        # Iteration t: V sweep (P banks) -> Q sweep + rope (W banks) ->
        # attention part A for q-tile t (old K/V chunks) with o_proj of
        # q-tile t-1 woven in -> K sweep + rope (W banks) -> diagonal
        # chunks -> fold/normalize.
        for t in range(NT):
            s0c = t * (ST1 // P)        # first V/K chunk index of this tile
            prefetch(t + 1)             # 36-deep x ring: safe this early
            po_it = po_emitter(t - 1)
            po_left = 16 if t >= 1 else 0

            def take_po(n):
                nonlocal po_left
                for _ in range(min(n, po_left)):
                    next(po_it)
                    po_left -= 1

            # --- V sweep into the even score tile (its last reader, an exp
            #     two chunks back, is long done); epilogue copies into V_sb
            for ht in range(NHT):
                for sub in range(ST1 // P):
                    nc.tensor.matmul(
                        SC4[0][:, sub * OLOC:(sub + 1) * OLOC],
                        lhsT=xfetch(t, ht)[:, sub * P:(sub + 1) * P],
                        rhs=wT_sb[:, ht, 2 * OLOC:3 * OLOC],
                        start=(ht == 0) and sub % 2 == 0,
                        stop=(ht == NHT - 1) and sub % 2 == 1,
                    )
            for sub in range(ST1 // P):
                nc.scalar.activation(
                    V_sb[:, s0c + sub, :],
                    SC4[0][:, sub * OLOC:(sub + 1) * OLOC],
                    mybir.ActivationFunctionType.Copy,
                )

            # --- Q sweep into Wa/Wb (head-major, rope muls under the other
            #     head's sweep); rope into QT_sb ---
            q12 = [None, None]
            for h in range(HPC):
                for ht in range(NHT):
                    nc.tensor.matmul(
                        W2[h],
                        lhsT=wT_sb[:, ht, h * P:(h + 1) * P],
                        rhs=xfetch(t, ht),
                        start=ht == 0, stop=ht == NHT - 1,
                    )
                q12[h] = rope_muls(t, "q", h, W2[h])
            rope_finish(t, 0, q12[0][0], q12[0][1], QT_sb)
            take_po(2)
            rope_finish(t, 1, q12[1][0], q12[1][1], QT_sb)
            take_po(2)

            # --- part A: chunks over already-built K/V, o_proj woven in ---
            cs = ChunkStream(t)
            for c in range(4 * t):
                cs.step()
                if po_left > 4:
                    take_po(2 if c % 2 == 0 else 1)

            # --- K sweep into Wa/Wb (head-major); rope into KT_sb ---
            take_po(po_left - 4)        # drain surplus o_proj, keep 4
            k12 = [None, None]
            for h in range(HPC):
                for ht in range(NHT):
                    nc.tensor.matmul(
                        W2[h],
                        lhsT=wT_sb[:, ht, OLOC + h * P:OLOC + (h + 1) * P],
                        rhs=xfetch(t, ht),
                        start=ht == 0, stop=ht == NHT - 1,
                    )
                k12[h] = rope_muls(t, "k", h, W2[h])
            cs.flush_pv()
            rope_finish(t, 0, k12[0][0], k12[0][1], KT_sb)
            cs.flush_pv()
            rope_finish(t, 1, k12[1][0], k12[1][1], KT_sb)
            cs.flush_pv()

            # --- part B: diagonal chunks, reserved o_proj as fill ---
            while cs.more:
                cs.step()
                take_po(1)
            cs.finish()
            take_po(po_left)
            cs.fold_normalize()

        # final o_proj for tile NT-1
        for _ in po_emitter(NT - 1):
            pass


def build_program():
    """Build + compile the Bass program (same program for all 8 cores)."""
    global _PROGRAM
    if _PROGRAM is not None:
        return _PROGRAM
    nc = bacc.Bacc(
        "TRN2", target_bir_lowering=False, debug=False, enable_asserts=False
    )
    with tile.TileContext(nc) as tc:
        _build_body(tc)
    nc.compile()
    _PROGRAM = nc
    return nc


def make_in_maps(hidden_states, w_qkv, w_o):
    import ml_dtypes

    x = np.asarray(hidden_states, dtype=np.float32).reshape(S, H)
    w = np.asarray(w_qkv, dtype=np.float32)
    wo = np.asarray(w_o, dtype=np.float32)

    xT = np.ascontiguousarray(x.T).astype(ml_dtypes.bfloat16)    # [2048, 4096]

    # rope tables, [128, 2, 4096]: rows 0:64 and 64:128 both hold the
    # [64, S] table so the doubled layout lines up with [real; imag] dims.
    e = np.arange(0, DH, 2, dtype=np.float32) / np.float32(DH)
    inv_freq = (1.0 / np.power(np.float32(10000.0), e)).astype(np.float32)
    t = np.arange(S, dtype=np.float32)
    freqs = np.outer(t, inv_freq).astype(np.float32)     # [S, 64]
    cosT = np.cos(freqs).T                               # [64, S]
    sinT = np.sin(freqs).T
    rope = np.empty((P, 2, S), dtype=np.float32)
    rope[0:64, 0] = cosT
    rope[64:128, 0] = cosT
    rope[0:64, 1] = sinT
    rope[64:128, 1] = sinT

    # signed half-swap permutation: (J.T @ z)[d] = -z[64+d], [64+d] = +z[d]
    swapj = np.zeros((P, P), dtype=ml_dtypes.bfloat16)
    for d in range(64):
        swapj[64 + d, d] = -1.0
        swapj[d, 64 + d] = 1.0

    # diagonal-block masks [4, 128, 1024]: chunk at k0 = q0 + 128j keeps
    # (ki, qi) iff qi >= ki + 128j; tiled twice along q for the 2-head tile.
    ki = np.arange(P)[:, None]
    qi = np.arange(ST1)[None, :]
    masks = np.empty((4, P, 2 * ST1), dtype=ml_dtypes.bfloat16)
    for j in range(4):
        m = (qi >= ki + 128 * j).astype(ml_dtypes.bfloat16)
        masks[j] = np.concatenate([m, m], axis=1)

    in_maps = []
    for c in range(NCORES):
        r0 = c * OLOC
        w_loc = np.concatenate(
            [
                w[r0:r0 + OLOC],
                w[NH * DH + r0:NH * DH + r0 + OLOC],
                w[2 * NH * DH + r0:2 * NH * DH + r0 + OLOC],
            ],
            axis=0,
        )                                                # [768, 2048]
        wqkvT_c = np.ascontiguousarray(w_loc.T).astype(ml_dtypes.bfloat16)
        woT_c = np.ascontiguousarray(
            wo[:, r0:r0 + OLOC].T
        ).astype(ml_dtypes.bfloat16)                     # [256, 2048]
        in_maps.append(
            {
                "xT": xT,
                "wqkvT": wqkvT_c,
                "woT": woT_c,
                "rope": rope,
                "swapj": swapj,
                "onesin": np.ones((P, P), dtype=np.float32),
                "masks": masks,
            }
        )
    return in_maps


def run_cores(in_maps, trace=False, **kwargs):
    nc = build_program()
    return run_bass_kernel_spmd(
        nc, in_maps, list(range(NCORES)), trace=trace, **kwargs
    )


def kernel(hidden_states, w_qkv, w_o):
    in_maps = make_in_maps(hidden_states, w_qkv, w_o)
    res = run_cores(in_maps)
    acc = res.results[0]["out"].astype(np.float32)
    for c in range(1, NCORES):
        acc = acc + res.results[c]["out"].astype(np.float32)
    return acc.reshape(1, S, H)
